# revision 5
# baseline (speedup 1.0000x reference)
"""TRN2 Bass/Tile kernel for nn_Loss_58317065945194.

Loss: per-sample EMD with r=2 over C=10 channels:
    d = p - q; S = cumsum(d, axis=1); out = mean_b sqrt(mean_c S^2)

Design (per core, B/8 = 262144 samples, fp8 host-quantized p,q):
  - 22 "units" of 1024 cols (unit 21: 512), 12 samples/col, 120 active
    partitions; the pq stream is padded to 128 DRAM rows so all 16 SDMA
    engines engage (~300-400 GB/s vs ~200 at 120 rows).
  - mm1: DoubleRow fp8 stationary (+L^T|-L^T) computes S = L(p-q) for 12
    samples/col straight into PSUM [128,2,512].
  - squares, split per-unit across three engines (ASSIGN):
      A: Act Square PSUM f32 -> fp8 sq (exact)
      D: DVE copy PSUM->f16 + DVE mult f16 (2x mode) -> f16 sq
      G/H: DVE copy PSUM->f16 + GpSimd mult -> fp8/f16 sq
  - mm3 (channel-sum): A/G units: ONE DoubleRow fp8 matmul with a
    per-position stationary w2d_i mapping (subtile jb, block b) ->
    row 24i+12jb+b, dense-packing 5 units (120 rows) into a single-bank
    psU tile. D/H units: two plain 32-col f16 matmuls via tile_position.
  - sqrt: one Act Sqrt(scale=1/C) + accum_out per psU tile (5 units);
    per-tile partial streams out early; host sums / B.
  - DMA: single sync HWDGE ring: 2 packed weight DMAs first (fast, so
    DMA semaphore lanes recycle promptly), then 11 uniform 2048-col
    chunks (dense descriptor queue sustains ~400 GB/s).
  - warmup: 36 dummy N=128 DR matmuls + early fillers lift the PE HAM
    clock gate (4/8 -> 8/8) just as the first chunk lands; uniform big
    chunks keep PE gaps short so the gate never re-throttles.
"""

import sys

import numpy as np

if "/opt/trn_rl_repo" not in sys.path:
    sys.path.insert(0, "/opt/trn_rl_repo")

import ml_dtypes

N_CORES = 8
B, C = 2097152, 10
BS = B // N_CORES            # 262144 samples/core
SPC = 12                     # samples per column
P = SPC * C                  # 120 partitions
NU = 22                      # units; 0-20 are 1024 cols, 21 is 512
UW = [1024] * 21 + [512]
F = sum(UW)                  # 22016 cols = 264192 slots (262144 + pad)
SPAD = F * SPC
NWARM = 36                   # dummy matmuls to pre-warm the PE HAM clock
SLACK = 3                                 # units between mm1 and its mm3 issue

# unit -> drain engine: A=Act square fp8; D=DVE copy+mult f16;
# G=DVE copy + GpSimd mult fp8; H=DVE copy + GpSimd mult f16
ASSIGN = list("GAGADAGAADAGADDAGAHDAA")
assert len(ASSIGN) == NU

# chunk schedule: (unit_list, ring) ring: 's'=sync HWDGE, 'g'=gpsimd SWDGE
CHUNKS = [
    ([0, 1], "s"), ([2, 3], "s"), ([4, 5], "s"), ([6, 7], "s"),
    ([8, 9], "s"), ([10, 11], "s"), ([12, 13], "s"), ([14, 15], "s"),
    ([16, 17], "s"), ([18, 19], "s"), ([20], "s"), ([21], "s"),
]
assert sorted(u for us, _ in CHUNKS for u in us) == list(range(NU))

# plain-mm3 stationary variants (f16, 32-col + tile_position), keyed
# (i, jb) = position-in-half, bank. Rows 24i+12jb..+12 must not straddle a
# 32-row col-group, so D/H units sit only at positions i in {0, 3, 4}.
WP_KEYS = [(3, 0), (3, 1), (4, 0), (4, 1)]

_cache = {}


def _build_program():
    import concourse.tile as tile
    from concourse import bacc, mybir

    f32, f16, f8 = mybir.dt.float32, mybir.dt.float16, mybir.dt.float8e4
    Alu = mybir.AluOpType
    Act = mybir.ActivationFunctionType
    DR = mybir.MatmulPerfMode.DoubleRow

    nc = bacc.Bacc(
        "TRN2", target_bir_lowering=False, debug=False, num_devices=N_CORES
    )
    pq_d = nc.dram_tensor("pq", [128, 2 * F], f8, kind="ExternalInput").ap()
    w8_d = nc.dram_tensor("w8", [P, 7, 2, 128], f8, kind="ExternalInput").ap()
    w2p_d = nc.dram_tensor(
        "w2p", [P, len(WP_KEYS), 32], f16, kind="ExternalInput"
    ).ap()
    o_d = nc.dram_tensor("partial", [128, 5], f32, kind="ExternalOutput").ap()

    unit_chunk = {}   # unit -> (chunk_idx, col offset within chunk)
    for ci, (us, _) in enumerate(CHUNKS):
        off = 0
        for u in us:
            unit_chunk[u] = (ci, off)
            off += UW[u]

    with tile.TileContext(nc) as tc:
        with (
            tc.tile_pool(name="const", bufs=1) as const,
            tc.tile_pool(name="io", bufs=1) as io,
            tc.tile_pool(name="sq", bufs=7) as sqp,
            tc.tile_pool(name="dump", bufs=2) as dump,
            tc.tile_pool(name="accp", bufs=1) as accp,
            tc.psum_pool(name="psS", bufs=3) as psS,
            tc.psum_pool(name="psU", bufs=2) as psU,
        ):
            w8 = const.tile([P, 7, 2, 128], f8)
            w2p = const.tile([P, len(WP_KEYS), 32], f16)
            wd = const.tile([P, 2, 128], f8)
            acc = accp.tile([128, 5], f32)
            w1 = w8[:, 0, :, :]
            w2pt = w8[:, 6, 0, :]
            # gpsimd: memsets only (its queue stays free for the multiplies)
            nc.gpsimd.memset(acc[:], 0.0)
            nc.gpsimd.memset(wd[:], 0.0)
            # weights: 2 packed DMAs at the HEAD of the sync ring; they
            # finish in <1us so the 8 DMA semaphore lanes recycle promptly
            # for the chunk stream (a slow weight DMA on another ring was
            # observed to stall chunk descriptor-gen on lane reuse)
            nc.sync.dma_start(w8[:], w8_d[:])
            nc.sync.dma_start(w2p[:], w2p_d[:])

            # Act: trigger both act-table loads ASAP via tiny warm acts
            warm = dump.tile([128, 2, 512], f16, tag="dmp", name="warmact")
            nc.scalar.activation(warm[0:1, 0, 0:3], acc[0:1, 0:3], Act.Square)
            nc.scalar.activation(warm[0:1, 0, 4:7], acc[0:1, 0:3], Act.Sqrt)

            # PE warmup: dummy DR matmuls on the zeroed wd tile (N=128);
            # must span >3.4us of PE busy to lift the HAM clock gate
            Sw = psS.tile([128, 2, 512], f32, tag="S2", name="Swarm")
            for _ in range(NWARM):
                nc.tensor.matmul(
                    Sw[:, 0, 0:128], wd[:], wd[:], start=True, stop=True,
                    perf_mode=DR,
                )

            # queue input chunks (program order = stream order)
            chunk_tiles = []
            col0 = 0
            for ci, (us, ring) in enumerate(CHUNKS):
                cw = sum(UW[u] for u in us)
                ct = io.tile([128, 2, cw], f8, tag=f"c{ci}", name=f"pq{ci}")
                eng = nc.sync if ring == "s" else nc.gpsimd
                eng.dma_start(ct[:], pq_d[:, 2 * col0 : 2 * (col0 + cw)])
                chunk_tiles.append(ct)
                col0 += cw

            # per-psU-half-tile mm3 accumulation bookkeeping (hf = 5-unit
            # group -> one single-bank [128,512] psU tile, rows 24i+12jb+b)
            psu_tiles = {}            # hf -> psU tile
            half_total = {}           # hf -> total matmuls expected
            half_seen = {}            # hf -> matmuls emitted so far
            for u in range(NU):
                hf = min(u // 5, 4)
                nmm = 1 if (ASSIGN[u] in "AG" and u != 21) else (
                    1 if u == 21 else 2
                )
                half_total[hf] = half_total.get(hf, 0) + nmm
            sq_tiles = {}

            def emit_unit(u):
                """mm1 pair + square drain for unit u."""
                ci, off = unit_chunk[u]
                ct = chunk_tiles[ci]
                uw = UW[u]
                nb = uw // 512
                S2 = psS.tile([128, 2, 512], f32, tag="S2", name=f"S2u{u}")
                for j in range(nb):
                    nc.tensor.matmul(
                        S2[:, j, :],
                        w1[:],
                        ct[0:P, :, off + 512 * j : off + 512 * (j + 1)],
                        start=True, stop=True, perf_mode=DR,
                    )
                eng = ASSIGN[u]
                dt = f8 if eng in "AG" else f16
                tg = "sq8" if eng in "AG" else "sq16"
                sq = sqp.tile([P, 2, 512], dt, tag=tg, name=f"sq{u}")
                if eng == "A":
                    nc.scalar.activation(
                        sq[0:P, 0:nb, :], S2[0:P, 0:nb, :], Act.Square
                    )
                else:
                    s16 = sqp.tile([P, 2, 512], f16, tag="s16", name=f"s16u{u}")
                    nc.vector.tensor_copy(s16[0:P, 0:nb, :], S2[0:P, 0:nb, :])
                    e = nc.vector if eng == "D" else nc.gpsimd
                    e.tensor_tensor(
                        sq[0:P, 0:nb, :], s16[0:P, 0:nb, :], s16[0:P, 0:nb, :],
                        Alu.mult,
                    )
                sq_tiles[u] = sq

            def emit_mm3(u):
                """mm3 for unit u into its psU tile + sqrt when tile done."""
                hf, i = min(u // 5, 4), u % 5
                if hf not in psu_tiles:
                    psu_tiles[hf] = psU.tile(
                        [128, 512], f32, tag="ss", name=f"ss{hf}"
                    )
                ss = psu_tiles[hf]
                sq = sq_tiles.pop(u)
                seen = half_seen.get(hf, 0)
                tot = half_total[hf]
                if u == 21:
                    # tail: plain fp8, rows 24-35; 512 cols only
                    nc.tensor.matmul(
                        ss[:, :], w2pt, sq[:, 0, :],
                        start=(seen == 0), stop=(seen + 1 == tot),
                    )
                    half_seen[hf] = seen + 1
                elif ASSIGN[u] in "AG":
                    nc.tensor.matmul(
                        ss[:, :], w8[:, 1 + i, :, :], sq[:, :, :],
                        start=(seen == 0), stop=(seen + 1 == tot),
                        perf_mode=DR,
                    )
                    half_seen[hf] = seen + 1
                else:
                    for jb in range(2):
                        wi = WP_KEYS.index((i, jb))
                        g32 = (24 * i + 12 * jb) // 32
                        nc.tensor.matmul(
                            ss[32 * g32 : 32 * g32 + 32, :],
                            w2p[:, wi, :], sq[:, jb, :],
                            start=(seen + jb == 0),
                            stop=(seen + jb + 1 == tot),
                            tile_position=(0, 32 * g32),
                        )
                    half_seen[hf] = seen + 2
                # sqrt once the whole psU tile is complete (mm3s are
                # emitted in unit order, so tile hf closes at its last unit)
                if u in (4, 9, 14, 19, 21):
                    rows = 120 if hf < 4 else 36
                    dmp = dump.tile([128, 2, 512], f16, tag="dmp")
                    nc.scalar.activation(
                        dmp[0:rows, 0, :], ss[0:rows, :], Act.Sqrt,
                        scale=1.0 / C, accum_out=acc[0:rows, hf : hf + 1],
                    )
                    nc.sync.dma_start(o_d[:, hf : hf + 1], acc[:, hf : hf + 1])

            GSLACK = 5
            pending = []

            def flush_mm3(now):
                while pending:
                    x = pending[0]
                    sl = GSLACK if ASSIGN[x] in "GH" else SLACK
                    if now - x >= sl:
                        emit_mm3(pending.pop(0))
                    else:
                        break

            for u in range(NU):
                emit_unit(u)
                if u < 4:
                    # HAM keep-alive fillers while the DMA stream ramps
                    for _ in range(2):
                        nc.tensor.matmul(
                            Sw[:, 0, 0:128], wd[:], wd[:], start=True,
                            stop=True, perf_mode=DR,
                        )
                pending.append(u)
                flush_mm3(u)
            flush_mm3(NU + GSLACK)
    nc.compile()
    return nc


def _weights():
    f8 = ml_dtypes.float8_e4m3
    w8 = np.zeros((P, 7, 2, 128), np.float32)
    w2p = np.zeros((P, len(WP_KEYS), 32), np.float32)
    for b in range(SPC):
        for cs in range(C):
            k = 10 * b + cs
            for co in range(C):
                if cs <= co:
                    w8[k, 0, 0, 10 * b + co] = 1.0
                    w8[k, 0, 1, 10 * b + co] = -1.0
            for i in range(5):
                w8[k, 1 + i, 0, 24 * i + b] = 1.0
                w8[k, 1 + i, 1, 24 * i + 12 + b] = 1.0
            w8[k, 6, 0, 24 + b] = 1.0
            for wi, (i, jb) in enumerate(WP_KEYS):
                r = 24 * i + 12 * jb + b
                w2p[k, wi, r - 32 * (r // 32)] = 1.0
    return w8.astype(f8), w2p.astype(np.float16)


def _make_in_maps(p, q):
    f8 = ml_dtypes.float8_e4m3
    p = np.asarray(p, dtype=np.float32).reshape(B, C)
    q = np.asarray(q, dtype=np.float32).reshape(B, C)
    w8, w2p = _weights()

    def lay(x):
        # [BS, C] -> padded [F, SPC, C] -> [SPC*C, F] = [120, F]
        xp = np.zeros((SPAD, C), np.float32)
        xp[:BS] = x
        return xp.reshape(F, SPC, C).transpose(1, 2, 0).reshape(P, F)

    # chunk column ranges in stream order
    widths = [sum(UW[u] for u in us) for us, _ in CHUNKS]
    in_maps = []
    for r in range(N_CORES):
        pp = lay(p[r * BS : (r + 1) * BS])
        qq = lay(q[r * BS : (r + 1) * BS])
        pq = np.zeros((128, 2 * F), np.float32)
        col0 = 0
        for cw in widths:
            pq[:P, 2 * col0 : 2 * col0 + cw] = pp[:, col0 : col0 + cw]
            pq[:P, 2 * col0 + cw : 2 * (col0 + cw)] = qq[:, col0 : col0 + cw]
            col0 += cw
        in_maps.append({"pq": pq.astype(f8), "w8": w8, "w2p": w2p})
    return in_maps


def kernel(p, q, r):
    assert int(r) == 2, f"kernel specialized for r=2, got {r}"
    if "nc" not in _cache:
        _cache["nc"] = _build_program()
    nc = _cache["nc"]

    in_maps = _make_in_maps(p, q)

    from concourse.bass_utils import run_bass_kernel_spmd

    res = run_bass_kernel_spmd(nc, in_maps, list(range(N_CORES)))
    total = 0.0
    for r_ in res.results:
        total += r_["partial"].astype(np.float64).sum()
    return np.float32(total / B)
